# revision 1
# baseline (speedup 1.0000x reference)
"""Fused MLA-with-GQA attention kernel for 8 Trainium2 NeuronCores.

Sharding: 8 cores = 2 (batch) x 4 (kv-head groups). Each core owns one
batch element, 4 query heads and 1 kv head (tensor parallel over heads),
with the kv_lora_rank (512) columns of Wqkv replicated. Each core
computes a partial output  attn_out_g @ Wo[rows_g]  and the host sums
the 4 group partials per batch element.

On-device layout is fully transposed (feature-major) so the whole chain
runs without any transposes:
  C1^T = (X @ W1)^T           lhsT=W1 tile,  rhs=X^T tile
  K^T  = (CKV @ Wk)^T         lhsT=Wk tile,  rhs=CKV^T tile
  V    = CKV @ Wv             lhsT=CKV^T[:, s-sub], rhs=Wv tile
  S^T[k,q] = (Q K^T)^T        lhsT=K^T[:, k-tile], rhs=Q^T
  den[*,q] = sum_k E^T[k,q]   lhsT=ones[128,128],  rhs=E^T  (sum+broadcast)
  O^T[dv,q] = sum_k V E^T     lhsT=V[k-tile],      rhs=E^T
  Y[s,n]  = sum_h O_h^T Wo_h  lhsT=O^T[:, s-sub],  rhs=Wo_h

All matmul operands are float32r (full-rate fp32-reduced), accumulation
in fp32 PSUM. Causal structure: k-tiles above the diagonal are skipped
entirely; diagonal k-tiles are computed on the column sub-range
[p:512] only, with a triangular mask multiply after exp.
"""

import math
import sys

import numpy as np

for _p in ("/opt/trn_rl_repo", "/root/.axon_site/_ro/trn_rl_repo"):
    if _p not in sys.path:
        try:
            import os

            if os.path.isdir(_p):
                sys.path.insert(0, _p)
        except Exception:
            pass

import concourse.bacc as bacc
import concourse.mybir as mybir
import concourse.tile as tile
from concourse.alu_op_type import AluOpType
from concourse.bass_utils import run_bass_kernel_spmd

# ---- problem constants (hardcoded; kernel.py must be self-contained) ----
HID = 2048
NH = 16
NKV = 4
NG = NH // NKV  # 4 q heads per kv head
LORA = 512
D_ROPE = 64
D_NOPE = 128
D_V = 128
D_QK = D_NOPE + D_ROPE  # 192
B, S = 2, 2048
ROPE_BASE = 10000.0
NCORES = 8

NHC = NG  # heads per core = 4
W1_COLS = NHC * D_QK + LORA  # 4*128 + 128 + 128 + 512 = 1280
SC = 512  # s-chunk width
NCHUNK = S // SC  # 4
KT = 128  # k tile
NKT_TOT = S // KT  # 16
SCALE = 1.0 / math.sqrt(D_QK)

F32 = mybir.dt.float32
F32R = mybir.dt.float32r
EXP = mybir.ActivationFunctionType.Exp

_PROGRAM_CACHE = {}


def _build_program(reps: int = 1):
    """reps>1 repeats the whole computation in one NEFF (for timing the
    marginal cost of one repetition, net of dispatch overhead)."""
    nc = bacc.Bacc("TRN2", target_bir_lowering=False, debug=False)

    xt_d = nc.dram_tensor("xt", [HID, S], F32, kind="ExternalInput").ap()
    w1_d = nc.dram_tensor("w1", [HID, W1_COLS], F32, kind="ExternalInput").ap()
    wk_d = nc.dram_tensor("wk", [LORA, D_QK], F32, kind="ExternalInput").ap()
    wv_d = nc.dram_tensor("wv", [LORA, D_V], F32, kind="ExternalInput").ap()
    wo_d = nc.dram_tensor("wo", [NHC * D_V, HID], F32, kind="ExternalInput").ap()
    cos_d = nc.dram_tensor("cosq", [128, S], F32, kind="ExternalInput").ap()
    sin_d = nc.dram_tensor("sinq", [128, S], F32, kind="ExternalInput").ap()
    tri_d = nc.dram_tensor("tri", [128, 128], F32, kind="ExternalInput").ap()
    y_d = nc.dram_tensor("y", [S, HID], F32, kind="ExternalOutput").ap()

    r = lambda ap: ap.bitcast(F32R)

    from contextlib import ExitStack

    with tile.TileContext(nc) as tc:
        with ExitStack() as ctx:
            constp = ctx.enter_context(tc.tile_pool(name="const", bufs=1))
            wop = ctx.enter_context(tc.tile_pool(name="wo", bufs=1))
            w1p = ctx.enter_context(tc.tile_pool(name="w1s", bufs=1))
            xp = ctx.enter_context(tc.tile_pool(name="x", bufs=1))
            qnp = ctx.enter_context(tc.tile_pool(name="qn", bufs=1))
            ckvp = ctx.enter_context(tc.tile_pool(name="ckv", bufs=1))
            kfp = ctx.enter_context(tc.tile_pool(name="kf", bufs=1))
            vp = ctx.enter_context(tc.tile_pool(name="v", bufs=1))
            ropep = ctx.enter_context(tc.tile_pool(name="rope", bufs=1))
            ep = ctx.enter_context(tc.tile_pool(name="e", bufs=3))
            onp = ctx.enter_context(tc.tile_pool(name="on", bufs=1))
            yp = ctx.enter_context(tc.tile_pool(name="y", bufs=2))
            mmp = ctx.enter_context(tc.tile_pool(name="mm", bufs=6, space="PSUM"))
            denp = ctx.enter_context(tc.tile_pool(name="den", bufs=1, space="PSUM"))
            op_ = ctx.enter_context(tc.tile_pool(name="o", bufs=1, space="PSUM"))
            # ---------------- constants ----------------
            tri_r = constp.tile([128, 128], F32R, tag="tri")
            nc.gpsimd.dma_start(tri_r[:], r(tri_d[:]))

            ones_f = constp.tile([128, 128], F32, tag="ones_f")
            nc.gpsimd.memset(ones_f[:], 1.0)
            ones_r = constp.tile([128, 128], F32R, tag="ones_r")
            nc.scalar.copy(ones_r[:], ones_f[:])

            # wk: 4 l-tiles [128, 192]; wv: 4 l-tiles [128, 128]
            wk_t = []
            wv_t = []
            for l in range(4):
                t = constp.tile([128, D_QK], F32R, tag=f"wk{l}")
                nc.gpsimd.dma_start(t[:], r(wk_d[128 * l : 128 * (l + 1), :]))
                wk_t.append(t)
                t = constp.tile([128, D_V], F32R, tag=f"wv{l}")
                nc.gpsimd.dma_start(t[:], r(wv_d[128 * l : 128 * (l + 1), :]))
                wv_t.append(t)

            # wo resident: per (head, n-block) moving tiles [128, 512]
            wo_t = [[None] * 4 for _ in range(NHC)]
            for h in range(NHC):
                for n in range(4):
                    t = wop.tile([128, 512], F32R, tag=f"wo{h}_{n}")
                    nc.gpsimd.dma_start(
                        t[:], r(wo_d[128 * h : 128 * (h + 1), 512 * n : 512 * (n + 1)])
                    )
                    wo_t[h][n] = t

            # persistent K / V state across chunks
            k_nope = kfp.tile([128, S], F32R, tag="k_nope")
            k_rope = kfp.tile([64, S], F32R, tag="k_rope")
            v_t = [
                vp.tile([128, D_V], F32R, tag=f"v{i}", name=f"v{i}")
                for i in range(NKT_TOT)
            ]

            for rep in range(reps):
              for c in range(NCHUNK):
                s0 = SC * c

                # ---------------- phase A: C1 = X @ W1 (transposed) --------
                x_t = []
                for ht in range(16):
                    t = xp.tile([128, SC], F32R, tag=f"x{ht}")
                    nc.sync.dma_start(
                        t[:], r(xt_d[128 * ht : 128 * (ht + 1), s0 : s0 + SC])
                    )
                    x_t.append(t)

                # w1 spans: per h-tile, five 256-col spans, double-buffered
                w1_t = {}
                for si in range(5):
                    for ht in range(16):
                        t = w1p.tile(
                            [128, 256], F32R, tag=f"w1_{ht}", bufs=2,
                            name=f"w1_{ht}_{si}_{c}",
                        )
                        # W1 stream split across both HWDGE rings (SP also
                        # carries X, so ACT takes the bigger share)
                        eng = nc.scalar if ht < 10 else nc.sync
                        eng.dma_start(
                            t[:],
                            r(w1_d[128 * ht : 128 * (ht + 1), 256 * si : 256 * (si + 1)]),
                        )
                        w1_t[(ht, si)] = t

                q_nope = []
                ckv_t = []
                qx1_ps = qx2_ps = None
                for j in range(10):
                    ps = mmp.tile([128, SC], F32, tag="mm")
                    for ht in range(16):
                        si, off = (j // 2, 128 * (j % 2))
                        nc.tensor.matmul(
                            ps[:],
                            w1_t[(ht, si)][:, off : off + 128],
                            x_t[ht][:],
                            start=(ht == 0),
                            stop=(ht == 15),
                        )
                    if j < 4:
                        t = qnp.tile([128, SC], F32R, tag=f"qn{j}")
                        nc.scalar.copy(t[:], ps[:])
                        q_nope.append(t)
                    elif j == 4:
                        qx1_ps = ps
                    elif j == 5:
                        qx2_ps = ps
                    else:
                        t = ckvp.tile([128, SC], F32R, tag=f"ckv{j - 6}")
                        nc.scalar.copy(t[:], ps[:])
                        ckv_t.append(t)

                # rope tables for this chunk
                cos_t = ropep.tile([128, SC], F32, tag="cos")
                nc.gpsimd.dma_start(cos_t[:], cos_d[:, s0 : s0 + SC])
                sin_t = ropep.tile([128, SC], F32, tag="sin")
                nc.gpsimd.dma_start(sin_t[:], sin_d[:, s0 : s0 + SC])

                # ---- Q rope (4 heads batched in 128 partitions) ----
                # All DVE ops full-tile (base partition 0); per-head row
                # extraction done with SB->SB DMAs (free to cross partitions).
                p1 = ropep.tile([128, SC], F32, tag="p1")
                t1 = ropep.tile([128, SC], F32, tag="t1")
                p2 = ropep.tile([128, SC], F32, tag="p2")
                t2 = ropep.tile([128, SC], F32, tag="t2")
                nc.vector.tensor_tensor(p1[:], qx1_ps[:], cos_t[:], AluOpType.mult)
                nc.vector.tensor_tensor(t1[:], qx2_ps[:], sin_t[:], AluOpType.mult)
                nc.vector.tensor_tensor(p2[:], qx2_ps[:], cos_t[:], AluOpType.mult)
                nc.vector.tensor_tensor(t2[:], qx1_ps[:], sin_t[:], AluOpType.mult)
                o1 = ropep.tile([128, SC], F32R, tag="o1")
                o2 = ropep.tile([128, SC], F32R, tag="o2")
                nc.vector.tensor_tensor(o1[:], p1[:], t1[:], AluOpType.subtract)
                nc.vector.tensor_tensor(o2[:], p2[:], t2[:], AluOpType.add)
                # rope_r[h]: head h rope rows [x1out(32); x2out(32)]
                rope_r = [
                    ropep.tile([64, SC], F32R, tag=f"rr{i}", name=f"rr{i}_{c}")
                    for i in range(NHC)
                ]
                for h in range(NHC):
                    sl = slice(32 * h, 32 * h + 32)
                    nc.gpsimd.dma_start(rope_r[h][0:32, :], o1[sl, :])
                    nc.gpsimd.dma_start(rope_r[h][32:64, :], o2[sl, :])

                # ---------------- K up-projection ----------------
                ps_kn = mmp.tile([128, SC], F32, tag="mm")
                for l in range(4):
                    nc.tensor.matmul(
                        ps_kn[:], wk_t[l][:, 0:128], ckv_t[l][:],
                        start=(l == 0), stop=(l == 3),
                    )
                nc.scalar.copy(k_nope[:, s0 : s0 + SC], ps_kn[:])

                ps_kr = mmp.tile([64, SC], F32, tag="mm")
                for l in range(4):
                    nc.tensor.matmul(
                        ps_kr[:], wk_t[l][:, 128:192], ckv_t[l][:],
                        start=(l == 0), stop=(l == 3),
                    )
                # K rope. kp = [x1*cos; x2*cos], kt = [x1*sin; x2*sin]
                # (cos/sin rows 0:32 == 32:64, so full-tile products work).
                # Swap kt halves via SB->SB DMA, then:
                #   k_rope[0:32]  = kp[0:32]  - kt_swap[0:32]   (= x1 cos - x2 sin)
                #   k_rope[32:64] = kp[32:64] + kt_swap[32:64]  (= x2 cos + x1 sin)
                kp = ropep.tile([64, SC], F32, tag="kp")
                kt_ = ropep.tile([64, SC], F32, tag="kt_")
                kts = ropep.tile([64, SC], F32, tag="kts")
                nc.vector.tensor_tensor(kp[:], ps_kr[:], cos_t[0:64, :], AluOpType.mult)
                nc.vector.tensor_tensor(kt_[:], ps_kr[:], sin_t[0:64, :], AluOpType.mult)
                nc.gpsimd.dma_start(kts[0:32, :], kt_[32:64, :])
                nc.gpsimd.dma_start(kts[32:64, :], kt_[0:32, :])
                nc.vector.tensor_tensor(
                    k_rope[0:32, s0 : s0 + SC], kp[0:32, :], kts[0:32, :],
                    AluOpType.subtract,
                )
                nc.vector.tensor_tensor(
                    k_rope[32:64, s0 : s0 + SC], kp[32:64, :], kts[32:64, :],
                    AluOpType.add,
                )

                # ---------------- V up-projection ----------------
                for ss in range(4):
                    ps_v = mmp.tile([128, D_V], F32, tag="mm")
                    for l in range(4):
                        nc.tensor.matmul(
                            ps_v[:],
                            ckv_t[l][:, 128 * ss : 128 * (ss + 1)],
                            wv_t[l][:],
                            start=(l == 0),
                            stop=(l == 3),
                        )
                    nc.scalar.copy(v_t[4 * c + ss][:], ps_v[:])

                # ---------------- phase B: attention per head ----------------
                o_norm = []
                for h in range(NHC):
                    nkt = 4 * c + 4
                    den_ps = denp.tile([128, SC], F32, tag="den")
                    o_ps = op_.tile([128, SC], F32, tag="o")
                    rr = rope_r[h]
                    for kt in range(nkt):
                        diag = kt >= 4 * c
                        p = (kt - 4 * c) * 128 if diag else 0
                        s_ps = mmp.tile([128, SC], F32, tag="mm")
                        nc.tensor.matmul(
                            s_ps[:, p:SC],
                            k_nope[:, KT * kt : KT * (kt + 1)],
                            q_nope[h][:, p:SC],
                            start=True,
                            stop=False,
                        )
                        nc.tensor.matmul(
                            s_ps[:, p:SC],
                            k_rope[:, KT * kt : KT * (kt + 1)],
                            rr[:, p:SC],
                            start=False,
                            stop=True,
                        )
                        e = ep.tile([128, SC], F32R, tag="e")
                        if diag:
                            tmp = ep.tile([128, 128], F32, tag="ediag", bufs=2,
                                          name=f"ediag_{c}_{h}_{kt}")
                            nc.scalar.activation(
                                tmp[:], s_ps[:, p : p + 128], EXP, scale=SCALE
                            )
                            nc.vector.tensor_tensor(
                                e[:, p : p + 128], tmp[:], tri_r[:], AluOpType.mult
                            )
                            if p + 128 < SC:
                                nc.scalar.activation(
                                    e[:, p + 128 : SC], s_ps[:, p + 128 : SC],
                                    EXP, scale=SCALE,
                                )
                        else:
                            nc.scalar.activation(e[:], s_ps[:], EXP, scale=SCALE)
                        nc.tensor.matmul(
                            den_ps[:, p:SC],
                            ones_r[:],
                            e[:, p:SC],
                            start=(kt == 0),
                            stop=(kt == nkt - 1),
                        )
                        nc.tensor.matmul(
                            o_ps[:, p:SC],
                            v_t[kt][:],
                            e[:, p:SC],
                            start=(kt == 0),
                            stop=(kt == nkt - 1),
                        )
                    recip = ropep.tile([128, SC], F32, tag="recip",
                                       name=f"recip_{c}_{h}")
                    nc.vector.reciprocal(recip[:], den_ps[:])
                    on = onp.tile([128, SC], F32R, tag=f"on{h}")
                    nc.vector.tensor_tensor(on[:], o_ps[:], recip[:], AluOpType.mult)
                    o_norm.append(on)

                # ---------------- phase C: Y = O @ Wo (partial) -------------
                # y staged [128, 1024] (2 n-blocks) -> bigger output DMAs,
                # issued on the SWDGE ring to keep the HWDGE rings free
                for ss in range(4):
                    for np_ in range(2):
                        y_sb = yp.tile([128, 1024], F32, tag="y", name=f"y_{c}_{ss}_{np_}")
                        for nn in range(2):
                            n = 2 * np_ + nn
                            y_ps = mmp.tile([128, 512], F32, tag="mm", name=f"yps_{c}_{ss}_{n}")
                            for h in range(NHC):
                                nc.tensor.matmul(
                                    y_ps[:],
                                    o_norm[h][:, 128 * ss : 128 * (ss + 1)],
                                    wo_t[h][n][:],
                                    start=(h == 0),
                                    stop=(h == NHC - 1),
                                )
                            nc.scalar.copy(y_sb[:, 512 * nn : 512 * (nn + 1)], y_ps[:])
                        nc.gpsimd.dma_start(
                            y_d[s0 + 128 * ss : s0 + 128 * (ss + 1),
                                1024 * np_ : 1024 * (np_ + 1)],
                            y_sb[:],
                        )

    nc.compile()
    return nc


def _host_inputs(hidden_states, Wqkv, Wk_up, Wv_up, Wo):
    """Build the 8 per-core input maps."""
    inv_freq = 1.0 / (ROPE_BASE ** (np.arange(0, D_ROPE, 2, dtype=np.float32) / D_ROPE))
    t = np.arange(S, dtype=np.float32)
    freqs = np.outer(t, inv_freq)  # [S, 32]
    cosq = np.ascontiguousarray(np.tile(np.cos(freqs).T, (4, 1))).astype(np.float32)
    sinq = np.ascontiguousarray(np.tile(np.sin(freqs).T, (4, 1))).astype(np.float32)
    tri = np.triu(np.ones((128, 128), dtype=np.float32))

    lora_cols = Wqkv[:, NH * D_QK :]  # [HID, LORA]
    in_maps = []
    per_g = {}
    for g in range(NKV):
        nopes, x1s, x2s = [], [], []
        for h in range(NHC):
            H = NHC * g + h
            base = H * D_QK
            nopes.append(Wqkv[:, base : base + D_NOPE])
            x1s.append(Wqkv[:, base + D_NOPE : base + D_NOPE + 32])
            x2s.append(Wqkv[:, base + D_NOPE + 32 : base + D_QK])
        w1 = np.ascontiguousarray(
            np.concatenate(nopes + x1s + x2s + [lora_cols], axis=1)
        ).astype(np.float32)
        wk = np.ascontiguousarray(
            np.concatenate(
                [
                    Wk_up[:, g * D_QK : g * D_QK + D_NOPE],
                    Wk_up[:, g * D_QK + D_NOPE : g * D_QK + D_NOPE + 32],
                    Wk_up[:, g * D_QK + D_NOPE + 32 : (g + 1) * D_QK],
                ],
                axis=1,
            )
        ).astype(np.float32)
        wv = np.ascontiguousarray(Wv_up[:, g * D_V : (g + 1) * D_V]).astype(np.float32)
        wo = np.ascontiguousarray(Wo[g * NHC * D_V : (g + 1) * NHC * D_V, :]).astype(
            np.float32
        )
        per_g[g] = (w1, wk, wv, wo)

    for core in range(NCORES):
        b, g = core // NKV, core % NKV
        w1, wk, wv, wo = per_g[g]
        xt = np.ascontiguousarray(hidden_states[b].T).astype(np.float32)
        in_maps.append(
            {
                "xt": xt,
                "w1": w1,
                "wk": wk,
                "wv": wv,
                "wo": wo,
                "cosq": cosq,
                "sinq": sinq,
                "tri": tri,
            }
        )
    return in_maps


def kernel(hidden_states, Wqkv, Wk_up, Wv_up, Wo):
    hidden_states = np.asarray(hidden_states, dtype=np.float32)
    Wqkv = np.asarray(Wqkv, dtype=np.float32)
    Wk_up = np.asarray(Wk_up, dtype=np.float32)
    Wv_up = np.asarray(Wv_up, dtype=np.float32)
    Wo = np.asarray(Wo, dtype=np.float32)

    if "nc" not in _PROGRAM_CACHE:
        _PROGRAM_CACHE["nc"] = _build_program()
    nc = _PROGRAM_CACHE["nc"]

    in_maps = _host_inputs(hidden_states, Wqkv, Wk_up, Wv_up, Wo)
    res = run_bass_kernel_spmd(nc, in_maps, list(range(NCORES)))

    out = np.zeros((B, S, HID), dtype=np.float32)
    for core in range(NCORES):
        b = core // NKV
        out[b] += res.results[core]["y"]
    return out


if __name__ == "__main__":
    rng = np.random.default_rng(0)
    hs = rng.standard_normal((B, S, HID)).astype(np.float32)
    wqkv = rng.standard_normal((HID, NH * D_QK + LORA)).astype(np.float32) * 0.02
    wk = rng.standard_normal((LORA, NKV * D_QK)).astype(np.float32) * 0.04
    wv = rng.standard_normal((LORA, NKV * D_V)).astype(np.float32) * 0.04
    wo = rng.standard_normal((NH * D_V, HID)).astype(np.float32) * 0.02
    y = kernel(hs, wqkv, wk, wv, wo)
    print("kernel output", y.shape, y.dtype, float(np.abs(y).max()))



# revision 3
# speedup vs baseline: 2.0481x; 2.0481x over previous
"""Fused MLA-with-GQA attention kernel for 8 Trainium2 NeuronCores.

Sharding: 8 cores = 2 (batch) x 4 (kv-head groups). Each core owns one
batch element, 4 query heads and 1 kv head (tensor parallel over heads),
with the kv_lora_rank (512) columns of Wqkv replicated. Each core
computes a partial output  attn_out_g @ Wo[rows_g]  and the host sums
the 4 group partials per batch element.

On-device layout is fully transposed (feature-major) so the whole chain
runs without any transposes:
  C1^T = (X @ W1)^T           lhsT=W1 tile,  rhs=X^T tile
  K^T  = (CKV @ Wk)^T         lhsT=Wk tile,  rhs=CKV^T tile
  V    = CKV @ Wv             lhsT=CKV^T[:, s-sub], rhs=Wv tile
  S^T[k,q] = (Q K^T)^T        lhsT=K^T[:, k-tile], rhs=Q^T
  den[*,q] = sum_k E^T[k,q]   lhsT=ones[128,128],  rhs=esum  (sum+broadcast)
  O^T[dv,q] = sum_k V E^T     lhsT=V[k-tile],      rhs=E^T
  Y[s,n]  = sum_h O_h^T Wo_h  lhsT=O^T[:, s-sub],  rhs=Wo_h

v2 notes:
 - all matmul operands bf16 (fp32 PSUM accumulation); rel err ~1e-3.
 - W1 / Wo / Wk / Wv / rope tables fully SBUF-resident (loaded once).
 - all PSUM evacuations on DVE (vector.tensor_copy), not ScalarE.
 - attention processes heads in PAIRS: the two 64-deep rope matmuls of a
   pair run concurrently in disjoint PE row-groups (k_rope duplicated on
   partitions 0-63 / 64-127, packed q-rope per pair).
 - softmax denominator: exp tiles pre-summed in groups of 4 on DVE, one
   ones-matmul per group instead of per k-tile.
"""

import math
import sys

import numpy as np

for _p in ("/opt/trn_rl_repo", "/root/.axon_site/_ro/trn_rl_repo"):
    if _p not in sys.path:
        try:
            import os

            if os.path.isdir(_p):
                sys.path.insert(0, _p)
        except Exception:
            pass

import concourse.bacc as bacc
import concourse.mybir as mybir
import concourse.tile as tile
from concourse.alu_op_type import AluOpType
from concourse.bass_utils import run_bass_kernel_spmd

# ---- problem constants (hardcoded; kernel.py must be self-contained) ----
HID = 2048
NH = 16
NKV = 4
NG = NH // NKV  # 4 q heads per kv head
LORA = 512
D_ROPE = 64
D_NOPE = 128
D_V = 128
D_QK = D_NOPE + D_ROPE  # 192
B, S = 2, 2048
ROPE_BASE = 10000.0
NCORES = 8

NHC = NG  # heads per core = 4
W1_COLS = NHC * D_QK + LORA  # 512 nope + 128 x1 + 128 x2 + 512 lora = 1280
SC = 512  # s-chunk width
NCHUNK = S // SC  # 4
KT = 128  # k tile
NKT_TOT = S // KT  # 16
SCALE = 1.0 / math.sqrt(D_QK)

F32 = mybir.dt.float32
BF16 = mybir.dt.bfloat16
EXP = mybir.ActivationFunctionType.Exp

_PROGRAM_CACHE = {}


def _build_program(reps: int = 1):
    """reps>1 repeats the whole computation in one NEFF (for timing the
    marginal cost of one repetition, net of dispatch overhead)."""
    nc = bacc.Bacc("TRN2", target_bir_lowering=False, debug=False)

    xt_d = nc.dram_tensor("xt", [HID, S], BF16, kind="ExternalInput").ap()
    w1_d = nc.dram_tensor("w1", [HID, W1_COLS], BF16, kind="ExternalInput").ap()
    wk_d = nc.dram_tensor("wk", [LORA, D_QK], BF16, kind="ExternalInput").ap()
    wv_d = nc.dram_tensor("wv", [LORA, D_V], BF16, kind="ExternalInput").ap()
    wo_d = nc.dram_tensor("wo", [NHC * D_V, HID], BF16, kind="ExternalInput").ap()
    cos_d = nc.dram_tensor("cosq", [128, S], BF16, kind="ExternalInput").ap()
    sin_d = nc.dram_tensor("sinq", [128, S], BF16, kind="ExternalInput").ap()
    tri_d = nc.dram_tensor("tri", [128, 128], BF16, kind="ExternalInput").ap()
    y_d = nc.dram_tensor("y", [S, HID], F32, kind="ExternalOutput").ap()

    from contextlib import ExitStack

    with tile.TileContext(nc) as tc:
        with ExitStack() as ctx:
            constp = ctx.enter_context(tc.tile_pool(name="const", bufs=1))
            wop = ctx.enter_context(tc.tile_pool(name="wo", bufs=1))
            w1p = ctx.enter_context(tc.tile_pool(name="w1s", bufs=1))
            xp = ctx.enter_context(tc.tile_pool(name="x", bufs=2))
            qnp = ctx.enter_context(tc.tile_pool(name="qn", bufs=1))
            ckvp = ctx.enter_context(tc.tile_pool(name="ckv", bufs=1))
            kfp = ctx.enter_context(tc.tile_pool(name="kf", bufs=1))
            vp = ctx.enter_context(tc.tile_pool(name="v", bufs=1))
            ropep = ctx.enter_context(tc.tile_pool(name="rope", bufs=1))
            rqp = ctx.enter_context(tc.tile_pool(name="rq", bufs=2))
            ep = ctx.enter_context(tc.tile_pool(name="e", bufs=3))
            esp = ctx.enter_context(tc.tile_pool(name="es", bufs=2))
            onp = ctx.enter_context(tc.tile_pool(name="on", bufs=1))
            yp = ctx.enter_context(tc.tile_pool(name="y", bufs=2))
            # PSUM: 2 + 2 + 2 + 2 = 8 banks
            pap = ctx.enter_context(tc.tile_pool(name="pa", bufs=2, space="PSUM"))
            sp = ctx.enter_context(tc.tile_pool(name="s", bufs=2, space="PSUM"))
            denp = ctx.enter_context(tc.tile_pool(name="den", bufs=2, space="PSUM"))
            op_ = ctx.enter_context(tc.tile_pool(name="o", bufs=2, space="PSUM"))

            # ---------------- constants (loaded once) ----------------
            tri_t = constp.tile([128, 128], BF16, tag="tri")
            nc.gpsimd.dma_start(tri_t[:], tri_d[:])

            ones_f = constp.tile([128, 128], F32, tag="ones_f")
            nc.gpsimd.memset(ones_f[:], 1.0)
            ones_t = constp.tile([128, 128], BF16, tag="ones_t")
            nc.vector.tensor_copy(ones_t[:], ones_f[:])

            wk_t = []
            wv_t = []
            for l in range(4):
                t = constp.tile([128, D_QK], BF16, tag=f"wk{l}")
                nc.gpsimd.dma_start(t[:], wk_d[128 * l : 128 * (l + 1), :])
                wk_t.append(t)
                t = constp.tile([128, D_V], BF16, tag=f"wv{l}")
                nc.gpsimd.dma_start(t[:], wv_d[128 * l : 128 * (l + 1), :])
                wv_t.append(t)

            # wo resident: per head [128, 2048]
            wo_t = []
            for h in range(NHC):
                t = wop.tile([128, HID], BF16, tag=f"wo{h}")
                nc.gpsimd.dma_start(t[:], wo_d[128 * h : 128 * (h + 1), :])
                wo_t.append(t)

            # w1 resident: 16 h-tiles [128, 1280]
            w1_t = []
            for ht in range(16):
                t = w1p.tile([128, W1_COLS], BF16, tag=f"w1_{ht}")
                eng = nc.scalar if ht % 2 else nc.sync
                eng.dma_start(t[:], w1_d[128 * ht : 128 * (ht + 1), :])
                w1_t.append(t)

            # rope tables resident (full length)
            cos_t = constp.tile([128, S], BF16, tag="cos")
            nc.gpsimd.dma_start(cos_t[:], cos_d[:])
            sin_t = constp.tile([128, S], BF16, tag="sin")
            nc.gpsimd.dma_start(sin_t[:], sin_d[:])

            # persistent K / V state across chunks
            k_nope = kfp.tile([128, S], BF16, tag="k_nope")
            # k_rope duplicated on both 64-partition halves for paired
            # row-group matmuls
            krx2 = kfp.tile([128, S], BF16, tag="krx2")
            v_t = [
                vp.tile([128, D_V], BF16, tag=f"v{i}", name=f"v{i}")
                for i in range(NKT_TOT)
            ]

            for rep in range(reps):
              for c in range(NCHUNK):
                s0 = SC * c
                csl = slice(s0, s0 + SC)

                # ---------------- phase A: C1 = X @ W1 (transposed) --------
                x_t = []
                for ht in range(16):
                    t = xp.tile([128, SC], BF16, tag=f"x{ht}", name=f"x{ht}_{c}_{rep}")
                    nc.sync.dma_start(
                        t[:], xt_d[128 * ht : 128 * (ht + 1), csl]
                    )
                    x_t.append(t)

                q_nope = []
                ckv_t = []
                qx1b = qx2b = None
                for j in range(10):
                    ps = pap.tile([128, SC], F32, tag="pa")
                    for ht in range(16):
                        nc.tensor.matmul(
                            ps[:],
                            w1_t[ht][:, 128 * j : 128 * (j + 1)],
                            x_t[ht][:],
                            start=(ht == 0),
                            stop=(ht == 15),
                        )
                    if j < 4:
                        t = qnp.tile([128, SC], BF16, tag=f"qn{j}")
                        nc.vector.tensor_copy(t[:], ps[:])
                        q_nope.append(t)
                    elif j == 4:
                        qx1b = ropep.tile([128, SC], BF16, tag="qx1b")
                        nc.vector.tensor_copy(qx1b[:], ps[:])
                    elif j == 5:
                        qx2b = ropep.tile([128, SC], BF16, tag="qx2b")
                        nc.vector.tensor_copy(qx2b[:], ps[:])
                    else:
                        t = ckvp.tile([128, SC], BF16, tag=f"ckv{j - 6}")
                        nc.vector.tensor_copy(t[:], ps[:])
                        ckv_t.append(t)

                # ---- Q rope (4 heads batched in 128 partitions) ----
                cosc = cos_t[:, csl]
                sinc = sin_t[:, csl]
                p1 = ropep.tile([128, SC], BF16, tag="p1")
                t1 = ropep.tile([128, SC], BF16, tag="t1")
                p2 = ropep.tile([128, SC], BF16, tag="p2")
                t2 = ropep.tile([128, SC], BF16, tag="t2")
                nc.vector.tensor_tensor(p1[:], qx1b[:], cosc, AluOpType.mult)
                nc.vector.tensor_tensor(t1[:], qx2b[:], sinc, AluOpType.mult)
                nc.vector.tensor_tensor(p2[:], qx2b[:], cosc, AluOpType.mult)
                nc.vector.tensor_tensor(t2[:], qx1b[:], sinc, AluOpType.mult)
                o1 = ropep.tile([128, SC], BF16, tag="o1")
                o2 = ropep.tile([128, SC], BF16, tag="o2")
                nc.vector.tensor_tensor(o1[:], p1[:], t1[:], AluOpType.subtract)
                nc.vector.tensor_tensor(o2[:], p2[:], t2[:], AluOpType.add)
                # rq[hp]: packed rope rows for head pair hp:
                #   [h0.x1out(32); h0.x2out(32); h1.x1out(32); h1.x2out(32)]
                rq = [
                    rqp.tile([128, SC], BF16, tag=f"rq{i}", name=f"rq{i}_{c}_{rep}")
                    for i in range(2)
                ]
                for hp in range(2):
                    for i in range(2):
                        h = 2 * hp + i
                        sl = slice(32 * h, 32 * h + 32)
                        nc.gpsimd.dma_start(rq[hp][64 * i : 64 * i + 32, :], o1[sl, :])
                        nc.gpsimd.dma_start(rq[hp][64 * i + 32 : 64 * i + 64, :], o2[sl, :])

                # ---------------- K up-projection ----------------
                ps_kn = pap.tile([128, SC], F32, tag="pa")
                for l in range(4):
                    nc.tensor.matmul(
                        ps_kn[:], wk_t[l][:, 0:128], ckv_t[l][:],
                        start=(l == 0), stop=(l == 3),
                    )
                nc.vector.tensor_copy(k_nope[:, csl], ps_kn[:])

                ps_kr = pap.tile([64, SC], F32, tag="pa")
                for l in range(4):
                    nc.tensor.matmul(
                        ps_kr[:], wk_t[l][:, 128:192], ckv_t[l][:],
                        start=(l == 0), stop=(l == 3),
                    )
                # K rope: kp = [x1*cos; x2*cos], kt_ = [x1*sin; x2*sin]
                # (cos/sin rows 0:32 == 32:64). Swap kt_ halves via SB->SB
                # DMA, combine, then duplicate rows 0:64 -> 64:128.
                krb = ropep.tile([64, SC], BF16, tag="krb")
                nc.vector.tensor_copy(krb[:], ps_kr[:])
                kp = ropep.tile([64, SC], BF16, tag="kp")
                kt_ = ropep.tile([64, SC], BF16, tag="kt_")
                kts = ropep.tile([64, SC], BF16, tag="kts")
                nc.vector.tensor_tensor(kp[:], krb[:], cos_t[0:64, csl], AluOpType.mult)
                nc.vector.tensor_tensor(kt_[:], krb[:], sin_t[0:64, csl], AluOpType.mult)
                nc.gpsimd.dma_start(kts[0:32, :], kt_[32:64, :])
                nc.gpsimd.dma_start(kts[32:64, :], kt_[0:32, :])
                nc.vector.tensor_tensor(
                    krx2[0:32, csl], kp[0:32, :], kts[0:32, :], AluOpType.subtract
                )
                nc.vector.tensor_tensor(
                    krx2[32:64, csl], kp[32:64, :], kts[32:64, :], AluOpType.add
                )
                nc.gpsimd.dma_start(krx2[64:128, csl], krx2[0:64, csl])

                # ---------------- V up-projection ----------------
                for ss in range(4):
                    ps_v = pap.tile([128, D_V], F32, tag="pa")
                    for l in range(4):
                        nc.tensor.matmul(
                            ps_v[:],
                            ckv_t[l][:, 128 * ss : 128 * (ss + 1)],
                            wv_t[l][:],
                            start=(l == 0),
                            stop=(l == 3),
                        )
                    nc.vector.tensor_copy(v_t[4 * c + ss][:], ps_v[:])

                # ---------------- phase B: attention per head pair ----------
                # PV matmuls are software-pipelined one k-tile behind the QK
                # matmuls so the PE (in-order) never waits on ACT's exp: while
                # exp(kt) runs, the PE does PV(kt-1) and can then start
                # QK(kt+1). Denominator matmuls (one per 4-k-tile group,
                # rhs = DVE-accumulated esum) are deferred the same way.
                nkt = 4 * c + 4
                ngrp = nkt // 4
                o_norm = [None] * NHC
                for hp in range(2):
                    h0, h1 = 2 * hp, 2 * hp + 1
                    den_ps = [
                        denp.tile([128, SC], F32, tag=f"den{i}", bufs=1,
                                  name=f"den{i}_{c}_{hp}_{rep}")
                        for i in range(2)
                    ]
                    o_ps = [
                        op_.tile([128, SC], F32, tag=f"o{i}", bufs=1,
                                 name=f"o{i}_{c}_{hp}_{rep}")
                        for i in range(2)
                    ]

                    pending = None  # (kt, p, [e0, e1], esum_or_None)

                    def emit_pending(pend):
                        kt_, p_, e_, es_ = pend
                        for i in range(2):
                            nc.tensor.matmul(
                                o_ps[i][:, p_:SC], v_t[kt_][:], e_[i][:, p_:SC],
                                start=(kt_ == 0), stop=(kt_ == nkt - 1),
                            )
                        if es_ is not None:
                            g_ = kt_ // 4
                            for i in range(2):
                                nc.tensor.matmul(
                                    den_ps[i][:], ones_t[:], es_[i][:],
                                    start=(g_ == 0), stop=(g_ == ngrp - 1),
                                )

                    for g in range(ngrp):
                        esum = [
                            esp.tile([128, SC], BF16, tag=f"es{i}", bufs=2,
                                     name=f"es{i}_{c}_{hp}_{g}_{rep}")
                            for i in range(2)
                        ]
                        for i4 in range(4):
                            kt = 4 * g + i4
                            diag = kt >= 4 * c
                            p = (kt - 4 * c) * 128 if diag else 0
                            ksl = slice(KT * kt, KT * (kt + 1))
                            s_ps = [
                                sp.tile([128, SC], F32, tag=f"s{i}", bufs=1,
                                        name=f"s{i}_{c}_{hp}_{kt}_{rep}")
                                for i in range(2)
                            ]
                            # nope matmuls (shared lhsT), then the two rope
                            # matmuls concurrently in row-groups 0-63/64-127
                            nc.tensor.matmul(
                                s_ps[0][:, p:SC], k_nope[:, ksl],
                                q_nope[h0][:, p:SC], start=True, stop=False,
                            )
                            nc.tensor.matmul(
                                s_ps[1][:, p:SC], k_nope[:, ksl],
                                q_nope[h1][:, p:SC], start=True, stop=False,
                            )
                            nc.tensor.matmul(
                                s_ps[0][:, p:SC], krx2[0:64, ksl],
                                rq[hp][0:64, p:SC], start=False, stop=True,
                            )
                            nc.tensor.matmul(
                                s_ps[1][:, p:SC], krx2[64:128, ksl],
                                rq[hp][64:128, p:SC], start=False, stop=True,
                            )
                            e_pair = []
                            for i in range(2):
                                e = ep.tile([128, SC], BF16, tag=f"e{i}",
                                            name=f"e{i}_{c}_{hp}_{kt}_{rep}")
                                if diag:
                                    tmp = ep.tile([128, 128], BF16, tag=f"ed{i}",
                                                  bufs=2,
                                                  name=f"ed{i}_{c}_{hp}_{kt}_{rep}")
                                    nc.scalar.activation(
                                        tmp[:], s_ps[i][:, p : p + 128], EXP,
                                        scale=SCALE,
                                    )
                                    nc.vector.tensor_tensor(
                                        e[:, p : p + 128], tmp[:], tri_t[:],
                                        AluOpType.mult,
                                    )
                                    if p + 128 < SC:
                                        nc.scalar.activation(
                                            e[:, p + 128 : SC],
                                            s_ps[i][:, p + 128 : SC],
                                            EXP, scale=SCALE,
                                        )
                                else:
                                    nc.scalar.activation(
                                        e[:], s_ps[i][:], EXP, scale=SCALE
                                    )
                                # DVE accumulate into esum (groups of 4)
                                if i4 == 0:
                                    nc.vector.tensor_copy(esum[i][:], e[:])
                                else:
                                    nc.vector.tensor_tensor(
                                        esum[i][:, p:SC], esum[i][:, p:SC],
                                        e[:, p:SC], AluOpType.add,
                                    )
                                e_pair.append(e)
                            if pending is not None:
                                emit_pending(pending)
                            pending = (kt, p, e_pair, esum if i4 == 3 else None)
                    emit_pending(pending)
                    pending = None

                    for i in range(2):
                        h = 2 * hp + i
                        recip = ropep.tile([128, SC], F32, tag=f"recip{i}",
                                           name=f"recip{i}_{c}_{hp}_{rep}")
                        nc.vector.reciprocal(recip[:], den_ps[i][:])
                        on = onp.tile([128, SC], BF16, tag=f"on{h}")
                        nc.vector.tensor_tensor(
                            on[:], o_ps[i][:], recip[:], AluOpType.mult
                        )
                        o_norm[h] = on

                # ---------------- phase C: Y = O @ Wo (partial) -------------
                for ss in range(4):
                    for np_ in range(2):
                        y_sb = yp.tile([128, 1024], F32, tag="y",
                                       name=f"y_{c}_{ss}_{np_}_{rep}")
                        for nn in range(2):
                            n = 2 * np_ + nn
                            y_ps = pap.tile([128, 512], F32, tag="pa",
                                            name=f"yps_{c}_{ss}_{n}_{rep}")
                            for h in range(NHC):
                                nc.tensor.matmul(
                                    y_ps[:],
                                    o_norm[h][:, 128 * ss : 128 * (ss + 1)],
                                    wo_t[h][:, 512 * n : 512 * (n + 1)],
                                    start=(h == 0),
                                    stop=(h == NHC - 1),
                                )
                            nc.vector.tensor_copy(
                                y_sb[:, 512 * nn : 512 * (nn + 1)], y_ps[:]
                            )
                        nc.gpsimd.dma_start(
                            y_d[s0 + 128 * ss : s0 + 128 * (ss + 1),
                                1024 * np_ : 1024 * (np_ + 1)],
                            y_sb[:],
                        )

    nc.compile()
    return nc


def _host_inputs(hidden_states, Wqkv, Wk_up, Wv_up, Wo):
    """Build the 8 per-core input maps (bf16 operands)."""
    import ml_dtypes

    bf = ml_dtypes.bfloat16
    inv_freq = 1.0 / (ROPE_BASE ** (np.arange(0, D_ROPE, 2, dtype=np.float32) / D_ROPE))
    t = np.arange(S, dtype=np.float32)
    freqs = np.outer(t, inv_freq)  # [S, 32]
    cosq = np.ascontiguousarray(np.tile(np.cos(freqs).T, (4, 1))).astype(bf)
    sinq = np.ascontiguousarray(np.tile(np.sin(freqs).T, (4, 1))).astype(bf)
    tri = np.triu(np.ones((128, 128), dtype=np.float32)).astype(bf)

    lora_cols = Wqkv[:, NH * D_QK :]  # [HID, LORA]
    in_maps = []
    per_g = {}
    for g in range(NKV):
        nopes, x1s, x2s = [], [], []
        for h in range(NHC):
            H = NHC * g + h
            base = H * D_QK
            nopes.append(Wqkv[:, base : base + D_NOPE])
            x1s.append(Wqkv[:, base + D_NOPE : base + D_NOPE + 32])
            x2s.append(Wqkv[:, base + D_NOPE + 32 : base + D_QK])
        w1 = np.ascontiguousarray(
            np.concatenate(nopes + x1s + x2s + [lora_cols], axis=1)
        ).astype(bf)
        wk = np.ascontiguousarray(
            np.concatenate(
                [
                    Wk_up[:, g * D_QK : g * D_QK + D_NOPE],
                    Wk_up[:, g * D_QK + D_NOPE : g * D_QK + D_NOPE + 32],
                    Wk_up[:, g * D_QK + D_NOPE + 32 : (g + 1) * D_QK],
                ],
                axis=1,
            )
        ).astype(bf)
        wv = np.ascontiguousarray(Wv_up[:, g * D_V : (g + 1) * D_V]).astype(bf)
        wo = np.ascontiguousarray(Wo[g * NHC * D_V : (g + 1) * NHC * D_V, :]).astype(bf)
        per_g[g] = (w1, wk, wv, wo)

    for core in range(NCORES):
        b, g = core // NKV, core % NKV
        w1, wk, wv, wo = per_g[g]
        xt = np.ascontiguousarray(hidden_states[b].T).astype(bf)
        in_maps.append(
            {
                "xt": xt,
                "w1": w1,
                "wk": wk,
                "wv": wv,
                "wo": wo,
                "cosq": cosq,
                "sinq": sinq,
                "tri": tri,
            }
        )
    return in_maps


def kernel(hidden_states, Wqkv, Wk_up, Wv_up, Wo):
    hidden_states = np.asarray(hidden_states, dtype=np.float32)
    Wqkv = np.asarray(Wqkv, dtype=np.float32)
    Wk_up = np.asarray(Wk_up, dtype=np.float32)
    Wv_up = np.asarray(Wv_up, dtype=np.float32)
    Wo = np.asarray(Wo, dtype=np.float32)

    if "nc" not in _PROGRAM_CACHE:
        _PROGRAM_CACHE["nc"] = _build_program()
    nc = _PROGRAM_CACHE["nc"]

    in_maps = _host_inputs(hidden_states, Wqkv, Wk_up, Wv_up, Wo)
    res = run_bass_kernel_spmd(nc, in_maps, list(range(NCORES)))

    out = np.zeros((B, S, HID), dtype=np.float32)
    for core in range(NCORES):
        b = core // NKV
        out[b] += res.results[core]["y"]
    return out


if __name__ == "__main__":
    rng = np.random.default_rng(0)
    hs = rng.standard_normal((B, S, HID)).astype(np.float32)
    wqkv = rng.standard_normal((HID, NH * D_QK + LORA)).astype(np.float32) * 0.02
    wk = rng.standard_normal((LORA, NKV * D_QK)).astype(np.float32) * 0.04
    wv = rng.standard_normal((LORA, NKV * D_V)).astype(np.float32) * 0.04
    wo = rng.standard_normal((NH * D_V, HID)).astype(np.float32) * 0.02
    y = kernel(hs, wqkv, wk, wv, wo)
    print("kernel output", y.shape, y.dtype, float(np.abs(y).max()))


# revision 9
# speedup vs baseline: 2.6876x; 1.3123x over previous
"""Fused MLA-with-GQA attention kernel for 8 Trainium2 NeuronCores.

Sharding: 8 cores = 2 (batch) x 4 (kv-head groups). Each core owns one
batch element, 4 query heads and 1 kv head (tensor parallel over heads).
The kv_lora projection (512 cols of Wqkv) is additionally split 4 ways
across the group cores and AllGathered on-device. Each core computes a
partial output  attn_out_g @ Wo[rows_g]  and the host sums the 4 group
partials per batch element.

On-device layout is fully transposed (feature-major) so the whole chain
runs without any transposes:
  C1^T = (X @ W1)^T           lhsT=W1 tile,  rhs=X^T tile
  K^T  = (CKV @ Wk)^T         lhsT=Wk tile,  rhs=CKV^T tile
  V    = CKV @ Wv             lhsT=CKV^T[:, s-sub], rhs=Wv tile
  S^T[k,q] = (Q K^T)^T        lhsT=K^T[:, k-tile], rhs=Q^T
  den[*,q] = sum_k E^T[k,q]   lhsT=ones[128,128],  rhs=esum  (sum+broadcast)
  O^T[dv,q] = sum_k V E^T     lhsT=V[k-tile],      rhs=E^T
  Y[s,n]  = sum_h O_h^T Wo_h  lhsT=O^T[:, s-sub],  rhs=Wo_h

Performance structure:
 - all matmul operands bf16 (fp32 PSUM accumulation); rel err ~4e-3.
 - W1 / Wo / Wk / Wv / rope tables fully SBUF-resident (loaded once).
 - all PSUM evacuations on DVE (vector.tensor_copy), not ScalarE.
 - attention processes heads in PAIRS: the two 64-deep rope matmuls of a
   pair run concurrently in disjoint PE row-groups (k_rope duplicated on
   partitions 0-63 / 64-127, packed q-rope per pair).
 - softmax denominator: exp tiles pre-summed in groups of 4 on DVE, one
   ones-matmul per group instead of per k-tile.
 - PV matmuls software-pipelined one k-tile behind QK so the in-order PE
   never waits on ACT's exp.
 - the lora block + AllGather for chunk c+1 are issued before chunk c's
   attention, hiding the collective entirely behind attention compute.
"""

import math
import sys

import numpy as np

for _p in ("/opt/trn_rl_repo", "/root/.axon_site/_ro/trn_rl_repo"):
    if _p not in sys.path:
        try:
            import os

            if os.path.isdir(_p):
                sys.path.insert(0, _p)
        except Exception:
            pass

import concourse.bacc as bacc
import concourse.mybir as mybir
import concourse.tile as tile
from concourse.alu_op_type import AluOpType
from concourse.bass_utils import run_bass_kernel_spmd

# ---- problem constants (hardcoded; kernel.py must be self-contained) ----
HID = 2048
NH = 16
NKV = 4
NG = NH // NKV  # 4 q heads per kv head
LORA = 512
D_ROPE = 64
D_NOPE = 128
D_V = 128
D_QK = D_NOPE + D_ROPE  # 192
B, S = 2, 2048
ROPE_BASE = 10000.0
NCORES = 8

NHC = NG  # heads per core = 4
# W1 per-core columns: 128 local-lora | 512 nope | 128 x1 | 128 x2.
LORA_LOC = LORA // NKV  # 128
W1_COLS = LORA_LOC + NHC * D_QK  # 896
REPLICA_GROUPS = [[0, 1, 2, 3], [4, 5, 6, 7]]
SC = 512  # s-chunk width
NCHUNK = S // SC  # 4
KT = 128  # k tile
NKT_TOT = S // KT  # 16
SCALE = 1.0 / math.sqrt(D_QK)

F32 = mybir.dt.float32
BF16 = mybir.dt.bfloat16
EXP = mybir.ActivationFunctionType.Exp

_PROGRAM_CACHE = {}


def _build_program(reps: int = 1):
    """reps>1 repeats the whole computation in one NEFF (for timing the
    marginal cost of one repetition, net of dispatch overhead)."""
    nc = bacc.Bacc("TRN2", target_bir_lowering=False, debug=False)

    xt_d = nc.dram_tensor("xt", [HID, S], BF16, kind="ExternalInput").ap()
    w1_d = nc.dram_tensor("w1", [HID, W1_COLS], BF16, kind="ExternalInput").ap()
    wk_d = nc.dram_tensor("wk", [LORA, D_QK], BF16, kind="ExternalInput").ap()
    wv_d = nc.dram_tensor("wv", [LORA, D_V], BF16, kind="ExternalInput").ap()
    wo_d = nc.dram_tensor("wo", [NHC * D_V, HID], BF16, kind="ExternalInput").ap()
    cos_d = nc.dram_tensor("cosq", [128, S], BF16, kind="ExternalInput").ap()
    sin_d = nc.dram_tensor("sinq", [128, S], BF16, kind="ExternalInput").ap()
    tri_d = nc.dram_tensor("tri", [128, 128], BF16, kind="ExternalInput").ap()
    y_d = nc.dram_tensor("y", [S, HID], F32, kind="ExternalOutput").ap()

    from contextlib import ExitStack

    with tile.TileContext(nc) as tc:
        with ExitStack() as ctx:
            constp = ctx.enter_context(tc.tile_pool(name="const", bufs=1))
            wop = ctx.enter_context(tc.tile_pool(name="wo", bufs=1))
            w1p = ctx.enter_context(tc.tile_pool(name="w1s", bufs=1))
            xp = ctx.enter_context(tc.tile_pool(name="x", bufs=2))
            qnp = ctx.enter_context(tc.tile_pool(name="qn", bufs=1))
            ckvp = ctx.enter_context(tc.tile_pool(name="ckv", bufs=2))
            dramp = ctx.enter_context(tc.tile_pool(name="dram", bufs=2, space="DRAM"))
            kfp = ctx.enter_context(tc.tile_pool(name="kf", bufs=1))
            vp = ctx.enter_context(tc.tile_pool(name="v", bufs=1))
            ropep = ctx.enter_context(tc.tile_pool(name="rope", bufs=1))
            rqp = ctx.enter_context(tc.tile_pool(name="rq", bufs=2))
            ep = ctx.enter_context(tc.tile_pool(name="e", bufs=3))
            esp = ctx.enter_context(tc.tile_pool(name="es", bufs=2))
            onp = ctx.enter_context(tc.tile_pool(name="on", bufs=1))
            yp = ctx.enter_context(tc.tile_pool(name="y", bufs=2))
            # PSUM: pa(2) + s0(1) + s1(1) + den0/1(2) + o0/1(2) = 8 banks
            pap = ctx.enter_context(tc.tile_pool(name="pa", bufs=2, space="PSUM"))
            sp = ctx.enter_context(tc.tile_pool(name="s", bufs=1, space="PSUM"))
            denp = ctx.enter_context(tc.tile_pool(name="den", bufs=1, space="PSUM"))
            op_ = ctx.enter_context(tc.tile_pool(name="o", bufs=1, space="PSUM"))

            # ---------------- constants (loaded once) ----------------
            tri_t = constp.tile([128, 128], BF16, tag="tri")
            nc.gpsimd.dma_start(tri_t[:], tri_d[:])

            ones_f = constp.tile([128, 128], F32, tag="ones_f")
            nc.gpsimd.memset(ones_f[:], 1.0)
            ones_t = constp.tile([128, 128], BF16, tag="ones_t")
            nc.vector.tensor_copy(ones_t[:], ones_f[:])

            wk_t = []
            wv_t = []
            for l in range(4):
                t = constp.tile([128, D_QK], BF16, tag=f"wk{l}")
                nc.gpsimd.dma_start(t[:], wk_d[128 * l : 128 * (l + 1), :])
                wk_t.append(t)
                t = constp.tile([128, D_V], BF16, tag=f"wv{l}")
                nc.gpsimd.dma_start(t[:], wv_d[128 * l : 128 * (l + 1), :])
                wv_t.append(t)

            # wo resident: per head [128, 2048]
            wo_t = []
            for h in range(NHC):
                t = wop.tile([128, HID], BF16, tag=f"wo{h}")
                nc.gpsimd.dma_start(t[:], wo_d[128 * h : 128 * (h + 1), :])
                wo_t.append(t)

            # w1 resident: 16 h-tiles [128, 896]
            w1_t = []
            for ht in range(16):
                t = w1p.tile([128, W1_COLS], BF16, tag=f"w1_{ht}")
                eng = nc.scalar if ht % 2 else nc.sync
                eng.dma_start(t[:], w1_d[128 * ht : 128 * (ht + 1), :])
                w1_t.append(t)

            # rope tables resident (full length)
            cos_t = constp.tile([128, S], BF16, tag="cos")
            nc.gpsimd.dma_start(cos_t[:], cos_d[:])
            sin_t = constp.tile([128, S], BF16, tag="sin")
            nc.gpsimd.dma_start(sin_t[:], sin_d[:])

            # persistent K / V state across chunks
            k_nope = kfp.tile([128, S], BF16, tag="k_nope")
            # k_rope duplicated on both 64-partition halves for paired
            # row-group matmuls
            krx2 = kfp.tile([128, S], BF16, tag="krx2")
            v_t = [
                vp.tile([128, D_V], BF16, tag=f"v{i}", name=f"v{i}")
                for i in range(NKT_TOT)
            ]

            def load_x(c, rep):
                tiles = []
                for ht in range(16):
                    t = xp.tile([128, SC], BF16, tag=f"x{ht}",
                                name=f"x{ht}_{c}_{rep}")
                    nc.sync.dma_start(
                        t[:], xt_d[128 * ht : 128 * (ht + 1), SC * c : SC * (c + 1)]
                    )
                    tiles.append(t)
                return tiles

            def emit_lora(c, rep, x_t):
                """Local 128-col lora projection + AllGather of c_kv^T."""
                ps = pap.tile([128, SC], F32, tag="pa", name=f"lora_{c}_{rep}")
                for ht in range(16):
                    nc.tensor.matmul(
                        ps[:], w1_t[ht][:, 0:128], x_t[ht][:],
                        start=(ht == 0), stop=(ht == 15),
                    )
                ckv_loc = ckvp.tile([128, SC], BF16, tag="ckv_loc",
                                    name=f"ckvloc_{c}_{rep}")
                nc.vector.tensor_copy(ckv_loc[:], ps[:])
                cc_in = dramp.tile([128, SC], BF16, tag="cc_in",
                                   name=f"cc_in_{c}_{rep}")
                cc_out = dramp.tile([LORA, SC], BF16, tag="cc_out",
                                    name=f"cc_out_{c}_{rep}")
                nc.sync.dma_start(cc_in[:], ckv_loc[:])
                nc.gpsimd.collective_compute(
                    "AllGather",
                    mybir.AluOpType.bypass,
                    replica_groups=REPLICA_GROUPS,
                    ins=[cc_in.opt()],
                    outs=[cc_out.opt()],
                )
                out = []
                for l in range(4):
                    t = ckvp.tile([128, SC], BF16, tag=f"ckv{l}",
                                  name=f"ckv{l}_{c}_{rep}")
                    nc.sync.dma_start(t[:], cc_out[128 * l : 128 * (l + 1), :])
                    out.append(t)
                return out

            seq = [(r, c) for r in range(reps) for c in range(NCHUNK)]
            x_cur = load_x(0, 0)
            ckv_cur = emit_lora(0, 0, x_cur)

            for idx, (rep, c) in enumerate(seq):
                s0 = SC * c
                csl = slice(s0, s0 + SC)
                x_t = x_cur
                ckv_t = ckv_cur

                # prefetch next iteration's X early (slot freed by idx-1)
                x_next = None
                if idx + 1 < len(seq):
                    rep2, c2 = seq[idx + 1]
                    x_next = load_x(c2, rep2)

                # ---------------- phase A: q blocks (j=1..6) ----------------
                q_nope = []
                qx1b = qx2b = None
                for j in range(1, 7):
                    ps = pap.tile([128, SC], F32, tag="pa",
                                  name=f"pa{j}_{c}_{rep}")
                    for ht in range(16):
                        nc.tensor.matmul(
                            ps[:],
                            w1_t[ht][:, 128 * j : 128 * (j + 1)],
                            x_t[ht][:],
                            start=(ht == 0),
                            stop=(ht == 15),
                        )
                    if j < 5:
                        t = qnp.tile([128, SC], BF16, tag=f"qn{j - 1}")
                        nc.vector.tensor_copy(t[:], ps[:])
                        q_nope.append(t)
                    elif j == 5:
                        qx1b = ropep.tile([128, SC], BF16, tag="qx1b")
                        nc.vector.tensor_copy(qx1b[:], ps[:])
                    else:
                        qx2b = ropep.tile([128, SC], BF16, tag="qx2b")
                        nc.vector.tensor_copy(qx2b[:], ps[:])

                # ---- Q rope (4 heads batched in 128 partitions) ----
                cosc = cos_t[:, csl]
                sinc = sin_t[:, csl]
                p1 = ropep.tile([128, SC], BF16, tag="p1")
                t1 = ropep.tile([128, SC], BF16, tag="t1")
                p2 = ropep.tile([128, SC], BF16, tag="p2")
                t2 = ropep.tile([128, SC], BF16, tag="t2")
                nc.vector.tensor_tensor(p1[:], qx1b[:], cosc, AluOpType.mult)
                nc.vector.tensor_tensor(t1[:], qx2b[:], sinc, AluOpType.mult)
                nc.vector.tensor_tensor(p2[:], qx2b[:], cosc, AluOpType.mult)
                nc.vector.tensor_tensor(t2[:], qx1b[:], sinc, AluOpType.mult)
                o1 = ropep.tile([128, SC], BF16, tag="o1")
                o2 = ropep.tile([128, SC], BF16, tag="o2")
                nc.vector.tensor_tensor(o1[:], p1[:], t1[:], AluOpType.subtract)
                nc.vector.tensor_tensor(o2[:], p2[:], t2[:], AluOpType.add)
                # rq[hp]: packed rope rows for head pair hp:
                #   [h0.x1out(32); h0.x2out(32); h1.x1out(32); h1.x2out(32)]
                rq = [
                    rqp.tile([128, SC], BF16, tag=f"rq{i}", name=f"rq{i}_{c}_{rep}")
                    for i in range(2)
                ]
                for hp in range(2):
                    for i in range(2):
                        h = 2 * hp + i
                        sl = slice(32 * h, 32 * h + 32)
                        nc.gpsimd.dma_start(rq[hp][64 * i : 64 * i + 32, :], o1[sl, :])
                        nc.gpsimd.dma_start(rq[hp][64 * i + 32 : 64 * i + 64, :], o2[sl, :])

                # ---------------- K up-projection ----------------
                ps_kn = pap.tile([128, SC], F32, tag="pa", name=f"kn_{c}_{rep}")
                for l in range(4):
                    nc.tensor.matmul(
                        ps_kn[:], wk_t[l][:, 0:128], ckv_t[l][:],
                        start=(l == 0), stop=(l == 3),
                    )
                nc.vector.tensor_copy(k_nope[:, csl], ps_kn[:])

                ps_kr = pap.tile([64, SC], F32, tag="pa", name=f"kr_{c}_{rep}")
                for l in range(4):
                    nc.tensor.matmul(
                        ps_kr[:], wk_t[l][:, 128:192], ckv_t[l][:],
                        start=(l == 0), stop=(l == 3),
                    )
                # K rope: kp = [x1*cos; x2*cos], kt_ = [x1*sin; x2*sin]
                # (cos/sin rows 0:32 == 32:64). Swap kt_ halves via SB->SB
                # DMA, combine, then duplicate rows 0:64 -> 64:128.
                krb = ropep.tile([64, SC], BF16, tag="krb")
                nc.vector.tensor_copy(krb[:], ps_kr[:])
                kp = ropep.tile([64, SC], BF16, tag="kp")
                kt_ = ropep.tile([64, SC], BF16, tag="kt_")
                kts = ropep.tile([64, SC], BF16, tag="kts")
                nc.vector.tensor_tensor(kp[:], krb[:], cos_t[0:64, csl], AluOpType.mult)
                nc.vector.tensor_tensor(kt_[:], krb[:], sin_t[0:64, csl], AluOpType.mult)
                nc.gpsimd.dma_start(kts[0:32, :], kt_[32:64, :])
                nc.gpsimd.dma_start(kts[32:64, :], kt_[0:32, :])
                nc.vector.tensor_tensor(
                    krx2[0:32, csl], kp[0:32, :], kts[0:32, :], AluOpType.subtract
                )
                nc.vector.tensor_tensor(
                    krx2[32:64, csl], kp[32:64, :], kts[32:64, :], AluOpType.add
                )
                nc.gpsimd.dma_start(krx2[64:128, csl], krx2[0:64, csl])

                # ---------------- V up-projection ----------------
                for ss in range(4):
                    ps_v = pap.tile([128, D_V], F32, tag="pa",
                                    name=f"v_{ss}_{c}_{rep}")
                    for l in range(4):
                        nc.tensor.matmul(
                            ps_v[:],
                            ckv_t[l][:, 128 * ss : 128 * (ss + 1)],
                            wv_t[l][:],
                            start=(l == 0),
                            stop=(l == 3),
                        )
                    nc.vector.tensor_copy(v_t[4 * c + ss][:], ps_v[:])

                # next iteration's lora block + AllGather: issued here so the
                # collective overlaps this chunk's attention.
                if idx + 1 < len(seq):
                    rep2, c2 = seq[idx + 1]
                    ckv_cur = emit_lora(c2, rep2, x_next)
                    x_cur = x_next

                # ---------------- phase B: attention per head pair ----------
                # PV matmuls are software-pipelined one k-tile behind the QK
                # matmuls so the PE (in-order) never waits on ACT's exp: while
                # exp(kt) runs, the PE does PV(kt-1) and can then start
                # QK(kt+1). Denominator matmuls (one per 4-k-tile group,
                # rhs = DVE-accumulated esum) are deferred the same way.
                nkt = 4 * c + 4
                ngrp = nkt // 4
                o_norm = [None] * NHC
                for hp in range(2):
                    h0, h1 = 2 * hp, 2 * hp + 1
                    den_ps = [
                        denp.tile([128, SC], F32, tag=f"den{i}", bufs=1,
                                  name=f"den{i}_{c}_{hp}_{rep}")
                        for i in range(2)
                    ]
                    o_ps = [
                        op_.tile([128, SC], F32, tag=f"o{i}", bufs=1,
                                 name=f"o{i}_{c}_{hp}_{rep}")
                        for i in range(2)
                    ]

                    pending = None  # (kt, p, [e0, e1], esum_or_None)

                    def emit_pending(pend):
                        kt_, p_, e_, es_ = pend
                        for i in range(2):
                            nc.tensor.matmul(
                                o_ps[i][:, p_:SC], v_t[kt_][:], e_[i][:, p_:SC],
                                start=(kt_ == 0), stop=(kt_ == nkt - 1),
                            )
                        if es_ is not None:
                            g_ = kt_ // 4
                            for i in range(2):
                                nc.tensor.matmul(
                                    den_ps[i][:], ones_t[:], es_[i][:],
                                    start=(g_ == 0), stop=(g_ == ngrp - 1),
                                )

                    for g in range(ngrp):
                        esum = [
                            esp.tile([128, SC], BF16, tag=f"es{i}", bufs=2,
                                     name=f"es{i}_{c}_{hp}_{g}_{rep}")
                            for i in range(2)
                        ]
                        for i4 in range(4):
                            kt = 4 * g + i4
                            diag = kt >= 4 * c
                            p = (kt - 4 * c) * 128 if diag else 0
                            ksl = slice(KT * kt, KT * (kt + 1))
                            s_ps = [
                                sp.tile([128, SC], F32, tag=f"s{i}", bufs=1,
                                        name=f"s{i}_{c}_{hp}_{kt}_{rep}")
                                for i in range(2)
                            ]
                            # nope matmuls (shared lhsT), then the two rope
                            # matmuls concurrently in row-groups 0-63/64-127
                            nc.tensor.matmul(
                                s_ps[0][:, p:SC], k_nope[:, ksl],
                                q_nope[h0][:, p:SC], start=True, stop=False,
                            )
                            nc.tensor.matmul(
                                s_ps[1][:, p:SC], k_nope[:, ksl],
                                q_nope[h1][:, p:SC], start=True, stop=False,
                            )
                            nc.tensor.matmul(
                                s_ps[0][:, p:SC], krx2[0:64, ksl],
                                rq[hp][0:64, p:SC], start=False, stop=True,
                            )
                            nc.tensor.matmul(
                                s_ps[1][:, p:SC], krx2[64:128, ksl],
                                rq[hp][64:128, p:SC], start=False, stop=True,
                            )
                            e_pair = []
                            for i in range(2):
                                e = ep.tile([128, SC], BF16, tag=f"e{i}",
                                            name=f"e{i}_{c}_{hp}_{kt}_{rep}")
                                if diag:
                                    tmp = ep.tile([128, 128], BF16, tag=f"ed{i}",
                                                  bufs=2,
                                                  name=f"ed{i}_{c}_{hp}_{kt}_{rep}")
                                    nc.scalar.activation(
                                        tmp[:], s_ps[i][:, p : p + 128], EXP,
                                        scale=SCALE,
                                    )
                                    nc.vector.tensor_tensor(
                                        e[:, p : p + 128], tmp[:], tri_t[:],
                                        AluOpType.mult,
                                    )
                                    if p + 128 < SC:
                                        nc.scalar.activation(
                                            e[:, p + 128 : SC],
                                            s_ps[i][:, p + 128 : SC],
                                            EXP, scale=SCALE,
                                        )
                                else:
                                    nc.scalar.activation(
                                        e[:], s_ps[i][:], EXP, scale=SCALE
                                    )
                                # DVE accumulate into esum (groups of 4)
                                if i4 == 0:
                                    nc.vector.tensor_copy(esum[i][:], e[:])
                                else:
                                    nc.vector.tensor_tensor(
                                        esum[i][:, p:SC], esum[i][:, p:SC],
                                        e[:, p:SC], AluOpType.add,
                                    )
                                e_pair.append(e)
                            if pending is not None:
                                emit_pending(pending)
                            pending = (kt, p, e_pair, esum if i4 == 3 else None)
                    emit_pending(pending)
                    pending = None

                    for i in range(2):
                        h = 2 * hp + i
                        recip = ropep.tile([128, SC], F32, tag=f"recip{i}",
                                           name=f"recip{i}_{c}_{hp}_{rep}")
                        nc.vector.reciprocal(recip[:], den_ps[i][:])
                        on = onp.tile([128, SC], BF16, tag=f"on{h}")
                        nc.vector.tensor_tensor(
                            on[:], o_ps[i][:], recip[:], AluOpType.mult
                        )
                        o_norm[h] = on

                # ---------------- phase C: Y = O @ Wo (partial) -------------
                for ss in range(4):
                    for np_ in range(2):
                        y_sb = yp.tile([128, 1024], F32, tag="y",
                                       name=f"y_{c}_{ss}_{np_}_{rep}")
                        for nn in range(2):
                            n = 2 * np_ + nn
                            y_ps = pap.tile([128, 512], F32, tag="pa",
                                            name=f"yps_{c}_{ss}_{n}_{rep}")
                            for h in range(NHC):
                                nc.tensor.matmul(
                                    y_ps[:],
                                    o_norm[h][:, 128 * ss : 128 * (ss + 1)],
                                    wo_t[h][:, 512 * n : 512 * (n + 1)],
                                    start=(h == 0),
                                    stop=(h == NHC - 1),
                                )
                            nc.vector.tensor_copy(
                                y_sb[:, 512 * nn : 512 * (nn + 1)], y_ps[:]
                            )
                        nc.sync.dma_start(
                            y_d[s0 + 128 * ss : s0 + 128 * (ss + 1),
                                1024 * np_ : 1024 * (np_ + 1)],
                            y_sb[:],
                        )

    nc.compile()
    return nc


def _host_inputs(hidden_states, Wqkv, Wk_up, Wv_up, Wo):
    """Build the 8 per-core input maps (bf16 operands)."""
    import ml_dtypes

    bf = ml_dtypes.bfloat16
    inv_freq = 1.0 / (ROPE_BASE ** (np.arange(0, D_ROPE, 2, dtype=np.float32) / D_ROPE))
    t = np.arange(S, dtype=np.float32)
    freqs = np.outer(t, inv_freq)  # [S, 32]
    cosq = np.ascontiguousarray(np.tile(np.cos(freqs).T, (4, 1))).astype(bf)
    sinq = np.ascontiguousarray(np.tile(np.sin(freqs).T, (4, 1))).astype(bf)
    tri = np.triu(np.ones((128, 128), dtype=np.float32)).astype(bf)

    lora_cols = Wqkv[:, NH * D_QK :]  # [HID, LORA]
    in_maps = []
    per_g = {}
    for g in range(NKV):
        nopes, x1s, x2s = [], [], []
        for h in range(NHC):
            H = NHC * g + h
            base = H * D_QK
            nopes.append(Wqkv[:, base : base + D_NOPE])
            x1s.append(Wqkv[:, base + D_NOPE : base + D_NOPE + 32])
            x2s.append(Wqkv[:, base + D_NOPE + 32 : base + D_QK])
        lora_loc = lora_cols[:, g * LORA_LOC : (g + 1) * LORA_LOC]
        w1 = np.ascontiguousarray(
            np.concatenate([lora_loc] + nopes + x1s + x2s, axis=1)
        ).astype(bf)
        wk = np.ascontiguousarray(
            np.concatenate(
                [
                    Wk_up[:, g * D_QK : g * D_QK + D_NOPE],
                    Wk_up[:, g * D_QK + D_NOPE : g * D_QK + D_NOPE + 32],
                    Wk_up[:, g * D_QK + D_NOPE + 32 : (g + 1) * D_QK],
                ],
                axis=1,
            )
        ).astype(bf)
        wv = np.ascontiguousarray(Wv_up[:, g * D_V : (g + 1) * D_V]).astype(bf)
        wo = np.ascontiguousarray(Wo[g * NHC * D_V : (g + 1) * NHC * D_V, :]).astype(bf)
        per_g[g] = (w1, wk, wv, wo)

    for core in range(NCORES):
        b, g = core // NKV, core % NKV
        w1, wk, wv, wo = per_g[g]
        xt = np.ascontiguousarray(hidden_states[b].T).astype(bf)
        in_maps.append(
            {
                "xt": xt,
                "w1": w1,
                "wk": wk,
                "wv": wv,
                "wo": wo,
                "cosq": cosq,
                "sinq": sinq,
                "tri": tri,
            }
        )
    return in_maps


def kernel(hidden_states, Wqkv, Wk_up, Wv_up, Wo):
    hidden_states = np.asarray(hidden_states, dtype=np.float32)
    Wqkv = np.asarray(Wqkv, dtype=np.float32)
    Wk_up = np.asarray(Wk_up, dtype=np.float32)
    Wv_up = np.asarray(Wv_up, dtype=np.float32)
    Wo = np.asarray(Wo, dtype=np.float32)

    if "nc" not in _PROGRAM_CACHE:
        _PROGRAM_CACHE["nc"] = _build_program()
    nc = _PROGRAM_CACHE["nc"]

    in_maps = _host_inputs(hidden_states, Wqkv, Wk_up, Wv_up, Wo)
    res = run_bass_kernel_spmd(nc, in_maps, list(range(NCORES)))

    out = np.zeros((B, S, HID), dtype=np.float32)
    for core in range(NCORES):
        b = core // NKV
        out[b] += res.results[core]["y"]
    return out


if __name__ == "__main__":
    rng = np.random.default_rng(0)
    hs = rng.standard_normal((B, S, HID)).astype(np.float32)
    wqkv = rng.standard_normal((HID, NH * D_QK + LORA)).astype(np.float32) * 0.02
    wk = rng.standard_normal((LORA, NKV * D_QK)).astype(np.float32) * 0.04
    wv = rng.standard_normal((LORA, NKV * D_V)).astype(np.float32) * 0.04
    wo = rng.standard_normal((NH * D_V, HID)).astype(np.float32) * 0.02
    y = kernel(hs, wqkv, wk, wv, wo)
    print("kernel output", y.shape, y.dtype, float(np.abs(y).max()))
